# revision 40
# baseline (speedup 1.0000x reference)
"""AgentCollisionLoss Trainium2 kernel.

Full inputs -> full output. Shards the N (sample) dim across 8 NeuronCores
(2 samples per core), computes the pairwise agent-collision loss on device,
and gathers the per-core (NL, B) losses into the full (B, N) output.

Device layout (per core):
  partition p = n_local*T + t            (104 rows)
  Inputs:  xina [P, 3B] f16 x-data (sync queue), consts [1, 416] f32 row
           broadcast to all partitions via gpsimd (vector queue DMA),
           xinb f16 + mcon f16 on the tensor queue.
  Stage A: world-frame disk centroids CXY [P, 2, B, D] f32.
  Stage B: outer-difference subs (DVE; one mid-size rect on gpsimd) ->
           squares (ACT) -> d2 adds (DVE for group 0 / gpsimd for group 1)
           into one packed f16 d2 tile, (pair, di, dj)-ordered.
  Min:     3-op tensor_tensor min tree over dj per group, then per-chunk
           strided reduces over di -> pdist [P, PP] f16.
  Pen:     sqrt (ACT; table pre-loaded via a dummy op), rr = dist*inv_pd,
           pen_neg = min(rr-1, 0) f16.
  Tail:    PE matmuls: S = wneg^T @ pen_neg (time-decay sum, weights
           negated and x64-scaled for f16), S^T via PE transpose, then
           loss[nl,i] = sum_q M[q,i] S[nl,q] with the host-built
           pair->agent incidence matrix M; mask-mul by moving/64; DMA out.
"""

import os
import sys

import numpy as np

for _p in ("/opt/trn_rl_repo", "/root/.axon_site/_ro/trn_rl_repo"):
    if os.path.isdir(_p) and _p not in sys.path:
        sys.path.insert(0, _p)

import bass_rust
import concourse.bass as bass
import concourse.mybir as mybir
import concourse.tile as tile
from concourse.bass_utils import run_bass_kernel_spmd
from concourse.vector_clock import ScopedClock


def _split_drain_and_barrier(self, tick_clock, wait_clock):
    """Kernel-tail drain, one semaphore per drain instruction.

    The walrus build in this container rejects instructions carrying more
    than one embedded sync wait ("Too many sync wait commands"). Tile's
    stock tail emits a single drain waiting on the full global clock, so
    split it: one drain per nonzero proc tick. add_sem_waits elides waits
    the engine has already observed, so each drain carries exactly one.
    """
    gc = list(tick_clock.global_clock)
    engs = [self.nc.sync, self.nc.vector, self.nc.scalar, self.nc.gpsimd,
            self.nc.tensor]
    nd = 0
    for idx, tick in enumerate(gc):
        if tick <= 0:
            continue
        v = [0] * len(gc)
        v[idx] = tick
        d = engs[nd % len(engs)].drain()
        nd += 1
        wait_clock.add_sem_waits(
            d.ins, ScopedClock({None: bass_rust.VectorClock(v)})
        )
    self.nc.all_engine_barrier()
    assert self.sems is not None
    popped = self.nc._tile_sem_poison_stack.pop()
    assert popped is self._sem_poison
    self.nc.clear_and_free_semaphores(list(self.sems.allocated().values()))
    self.nc.all_engine_barrier()


tile.TileContext._drain_and_barrier = _split_drain_and_barrier

B, N, T, D = 32, 16, 52, 5
NCORES = 8
NL = N // NCORES          # samples per core
P = NL * T                # partition rows per core
BUFFER_DIST = 0.2
DECAY_RATE = 0.9
SPEED_TH = 0.5
WSCALE = 64.0             # f16-normal range scaling for the decay weights

F32 = mybir.dt.float32
F16 = mybir.dt.float16
PI = float(np.pi)

DT_BULK = F16
# rect-A chunks (from tree group 1) whose subs go to gpsimd, by F range
SUB_GP_MIN_F = 400
SUB_GP_MAX_F = 700


def _rects(scenes):
    """Circulant half-pair rects per scene, largest first.

    Each unordered same-scene pair {i, j} is covered exactly once:
    rect A: (i, k) for i in [0,s), k in [1,K], j = (i+k) mod s, K=(s-1)//2
    rect B (even s): (i, s/2) for i in [0, s/2), j = i + s/2
    Returns [(o, s, K, half)] with half = s//2 if s even else 0.
    """
    out = []
    for (o, s) in scenes:
        K = (s - 1) // 2
        half = s // 2 if s % 2 == 0 else 0
        out.append((o, s, K, half))
    out.sort(key=lambda r: -(r[1] * r[2] + r[3]))
    return out


# xinA (f16, replicated consts carry f32 islands as bitcast pairs):
# x0(B) x1(B) yaw(B) | gA(2B) gB(2B) | gT(2B f32 = 4B slots) |
# shifts2(2B f32 = 4B slots) | cent(B*D)
XO_GA = 3 * B
XO_GT = XO_GA + 4 * B          # f32-bitcast island
XO_SH = XO_GT + 4 * B          # f32-bitcast island
XO_CE = XO_SH + 4 * B
XWA = XO_CE + B * D
# xinB (f16): movt/WSCALE(B) | w*WSCALE(NL) | ident(NL) | inv_pd^2(PP)
XO_MVR = 0
XO_W = XO_MVR + B
XO_ID = XO_W + NL
XO_PRC = XO_ID + NL


def _xin_width_b(PP):
    return XO_PRC + PP


def _build_nc(scenes, PP):
    """Build the SPMD Bass program. `scenes` = [(offset, size)], PP = #pairs."""
    nc = bass.Bass()
    assert PP <= 128

    XWB = _xin_width_b(PP)
    xina = nc.dram_tensor("xina", [P, XWA], F16, kind="ExternalInput")
    xinb = nc.dram_tensor("xinb", [P, XWB], F16, kind="ExternalInput")
    mcon = nc.dram_tensor("mcon", [PP, B], F16, kind="ExternalInput")
    out = nc.dram_tensor("loss", [NL, B], F32, kind="ExternalOutput")

    rects = _rects(scenes)

    # chunk list: (kind, rect-idx, o, s, K, half, q-offset, q-count, F)
    # pair/q order: per rect (A-pairs (i,k) i-major, then its half pairs)
    chunks = []
    po = 0
    for idx, (o, s, K, half) in enumerate(rects):
        if K >= 1:
            chunks.append(("A", idx, o, s, K, half, po, s * K, s * K * D * D))
        if half:
            chunks.append(("H", idx, o, s, K, half, po + s * K, half,
                           half * D * D))
        po += s * K + half
    assert po == PP

    # d2 segment offsets (in (pair,di) units of 5 dj elems). Group
    # boundaries are padded to EVEN unit counts: the HW 2x reduce path
    # needs 4-byte-aligned f16 starts and even element counts.
    seg_off = {}
    pad_units = []
    so = 0
    prev_grp = None
    for c in chunks:
        g = 0 if c[1] <= 1 else 1
        if prev_grp is not None and g != prev_grp and so % 2:
            pad_units.append(so)
            so += 1
        prev_grp = g
        seg_off[id(c)] = so
        so += c[8] // D
    if so % 2:
        pad_units.append(so)
        so += 1
    NSEG = so

    # two pipeline groups: group 0 = the two largest rects (d2 adds on DVE),
    # group 1 = the rest (d2 adds on gpsimd). Groups are contiguous q ranges.
    grp_of = {}
    for c in chunks:
        grp_of[id(c)] = 0 if c[1] <= 1 else 1
    g_chunks = [[c for c in chunks if grp_of[id(c)] == g] for g in (0, 1)]
    g_chunks = [g for g in g_chunks if g]
    g_qr = []
    for g in g_chunks:
        q0 = min(c[6] for c in g)
        q1 = max(c[6] + c[7] for c in g)
        g_qr.append((q0, q1))

    # gpsimd measures 2.4-12 ns/elem on HW (vs DVE 0.59-1.1) — keep ALL
    # stage-B work on DVE; gpsimd stays idle.
    gp_subs = set()

    with tile.TileContext(nc) as tc:
        with (
            tc.tile_pool(name="singles", bufs=1) as singles,
            tc.tile_pool(name="small", bufs=1) as small,
            tc.tile_pool(name="big", bufs=1) as big,
            tc.tile_pool(name="psum", bufs=1, space="PSUM") as psum,
        ):
            # ---- loads: xina on sync, consts on vector, xinb+mcon on
            # tensor, so the three DIRECT2D descriptor-gen steps overlap ----
            xta = singles.tile([P, XWA], F16)
            nc.sync.dma_start(out=xta[:], in_=xina[:])
            # xinb + mcon issue from the ACT queue: the sync queue then has
            # only xina ahead of it (lands earliest), and ACT's early idle
            # window absorbs the ~0.7us descriptor-gen per DMA.
            xtb = singles.tile([P, XWB], F16)
            nc.scalar.dma_start(out=xtb[:], in_=xinb[:])
            mct = singles.tile([PP, B], F16)
            nc.scalar.dma_start(out=mct[:], in_=mcon[:])

            ones = singles.tile([P, 1], F32)
            nc.vector.memset(ones[:], 1.0)

            # Pre-touch xta on DVE: carries the xina DMA-queue wait so all
            # later DVE readers of xta need no DMA wait of their own.
            tch = singles.tile([P, 1], F16, tag="tch0")
            nc.vector.tensor_copy(out=tch[:], in_=xta[:, 0:1])

            gA = xta[:, XO_GA : XO_GA + 2 * B]
            gT = xta[:, XO_GT : XO_GT + 4 * B].bitcast(F32)
            shifts2 = xta[:, XO_SH : XO_SH + 4 * B].bitcast(F32)
            cxc = xta[:, XO_CE : XO_CE + B * D]
            x0 = xta[:, 0:B]
            yw = xta[:, 2 * B : 3 * B]
            movt = xtb[0:NL, XO_MVR : XO_MVR + B]   # replicated const rows
            prc2 = xtb[:, XO_PRC : XO_PRC + PP]     # inv_pd^2 per pair

            def rep2(apx, w):
                """view [P, 2, w] reading apx's first w elems twice"""
                return bass.AP(tensor=apx.tensor, offset=apx.offset,
                               ap=[apx.ap[0], [0, 2], [1, w]])

            # ---- stage A ----
            # u = yaw/2pi + (shift + yoff/2pi)   (shift 2.0 -> sin, 2.25 -> cos)
            u2 = small.tile([P, 2, B], F32)
            nc.vector.scalar_tensor_tensor(
                out=u2[:], in0=rep2(yw, B), scalar=1.0 / (2.0 * PI),
                in1=shifts2.rearrange("p (c i) -> p c i", c=2),
                op0=mybir.AluOpType.mult, op1=mybir.AluOpType.add)
            # round-to-nearest-even via the 1.5*2^23 magic constant
            MAGIC = 12582912.0
            kf = small.tile([P, 2, B], F32)
            nc.vector.tensor_scalar(
                out=kf[:], in0=u2[:], scalar1=MAGIC, scalar2=MAGIC,
                op0=mybir.AluOpType.add, op1=mybir.AluOpType.subtract)
            fr = small.tile([P, 2, B], F32)
            nc.vector.tensor_sub(fr[:], u2[:], kf[:])
            # sincos[:, 0:32] = sin(yawg), [:, 32:64] = cos(yawg)
            sincos = small.tile([P, 2 * B], F32)
            nc.scalar.activation(out=sincos[:].rearrange("p (c i) -> p c i", c=2),
                                 in_=fr[:],
                                 func=mybir.ActivationFunctionType.Sin,
                                 bias=0.0, scale=2.0 * PI)
            # dummy sqrt so the Sqrt table load happens right after sin in
            # the idle-ACT window instead of on the critical tail. Reading
            # sincos (not ones) keeps the scheduler from hoisting it (and
            # its table load) in front of sin.
            dum = small.tile([P, 1], F32, tag="dum")
            nc.scalar.activation(out=dum[:], in_=sincos[:, 0:1],
                                 func=mybir.ActivationFunctionType.Sqrt)

            # pos_g for both coords: pg[p, c, i], c=0 -> x, 1 -> y
            # m12[p, xsel, c, i] = x_xsel * g_{xsel,c}  in one multiply
            m12 = small.tile([P, 2, 2, B], F32)
            xx = bass.AP(tensor=xta.tensor, offset=x0.offset,
                         ap=[x0.ap[0], [B, 2], [0, 2], [1, B]])
            gAB = bass.AP(tensor=xta.tensor, offset=gA.offset,
                          ap=[gA.ap[0], [2 * B, 2], [B, 2], [1, B]])
            nc.vector.tensor_mul(m12[:], xx, gAB)
            pg = small.tile([P, 2, B], F32)
            nc.vector.tensor_add(pg[:], m12[:, 0], m12[:, 1])
            nc.vector.tensor_add(pg[:], pg[:],
                                 gT.rearrange("p (c i) -> p c i", c=2))

            # CXY[p, c, i, di] = cent_x(i,di) * cs(c,i) + pg(c,i)
            cxy = singles.tile([P, 2, B, D], F32)
            cs_sel = bass.AP(tensor=sincos.tensor, offset=sincos[:].offset + B,
                             ap=[sincos[:].ap[0], [-B, 2], [1, B], [0, D]])
            cx_rep = bass.AP(tensor=xta.tensor, offset=cxc.offset,
                             ap=[cxc.ap[0], [0, 2], [D, B], [1, D]])
            pg_bc = bass.AP(tensor=pg.tensor, offset=pg[:].offset,
                            ap=[pg[:].ap[0], [B, 2], [1, B], [0, D]])
            nc.vector.tensor_mul(cxy[:], cx_rep, cs_sel)
            nc.vector.tensor_add(cxy[:], cxy[:], pg_bc)

            cxyf = cxy[:].rearrange("p c i d -> p (c i d)")
            pap = cxyf.ap[0]
            e = cxyf.ap[-1][0]
            NPTS = B * D

            # doubled per-scene point lists (wrap j = (i+k) mod s becomes a
            # linear read). Kept on DVE: subs read cxy AND cxy2 and may
            # carry only one sync wait, so both must be DVE-written.
            DBL = 2 * NPTS
            cxy2 = singles.tile([P, 2, DBL], F32)
            c2f = cxy2[:].rearrange("p c d -> p (c d)")
            pap2 = c2f.ap[0]
            e2 = c2f.ap[-1][0]
            dbl_off = {}
            do_ = 0
            for (o, s) in scenes:
                dbl_off[o] = do_
                do_ += 2 * D * s

            subx = big.tile([P, NSEG * D], DT_BULK, tag="subx")
            suby = big.tile([P, NSEG * D], DT_BULK, tag="suby")
            sqx = big.tile([P, NSEG * D], DT_BULK, tag="sqx")
            sqy = big.tile([P, NSEG * D], DT_BULK, tag="sqy")
            d2 = big.tile([P, NSEG * D], DT_BULK, tag="d2")
            subt = {0: subx, 1: suby}
            sqt = {0: sqx, 1: sqy}
            for pu in pad_units:
                nc.vector.memset(d2[:, pu * D : (pu + 1) * D], 60000.0)

            def seg_ap(t, off_e, F):
                tf = t[:]
                es = tf.ap[-1][0]
                return bass.AP(tensor=tf.tensor, offset=tf.offset + off_e * es,
                               ap=[tf.ap[0], [es, F]])

            def emit_dbl_copy(o, s, eng):
                # each scene's doubled block is built by the engine that
                # subs it, so the subs see a single-engine cxy/cxy2 pair
                # (DVE) or carry one DVE wait (gpsimd)
                in_ap = bass.AP(tensor=cxyf.tensor,
                                offset=cxyf.offset + o * D * e,
                                ap=[pap, [NPTS * e, 2], [0, 2], [e, D * s]])
                out_ap = bass.AP(tensor=c2f.tensor,
                                 offset=c2f.offset + dbl_off[o] * e2,
                                 ap=[pap2, [DBL * e2, 2], [D * s * e2, 2],
                                     [e2, D * s]])
                eng.tensor_copy(out=out_ap, in_=in_ap)

            def emit_subs(c):
                kind, idx, o, s, K, half, qoff, qn, F = c
                off_e = seg_off[id(c)] * D
                eng = nc.gpsimd if id(c) in gp_subs else nc.vector
                if kind == "A":
                    w = D * K
                    for cc in range(2):
                        a_ap = bass.AP(
                            tensor=cxyf.tensor,
                            offset=cxyf.offset + (cc * NPTS + o * D) * e,
                            ap=[pap, [D * e, s], [e, D], [0, w]])
                        b_ap = bass.AP(
                            tensor=c2f.tensor,
                            offset=c2f.offset + (cc * DBL + dbl_off[o] + D) * e2,
                            ap=[pap2, [D * e2, s], [0, D], [e2, w]])
                        eng.tensor_tensor(
                            out=seg_ap(subt[cc], off_e, F), in0=a_ap,
                            in1=b_ap, op=mybir.AluOpType.subtract)
                else:
                    for cc in range(2):
                        a_ap = bass.AP(
                            tensor=cxyf.tensor,
                            offset=cxyf.offset + (cc * NPTS + o * D) * e,
                            ap=[pap, [D * e, half], [e, D], [0, D]])
                        b_ap = bass.AP(
                            tensor=cxyf.tensor,
                            offset=cxyf.offset + (cc * NPTS + (o + half) * D) * e,
                            ap=[pap, [D * e, half], [0, D], [e, D]])
                        eng.tensor_tensor(
                            out=seg_ap(subt[cc], off_e, F), in0=a_ap,
                            in1=b_ap, op=mybir.AluOpType.subtract)

            # interleave the cxy2 doubling copies with the subs, rect order
            done_dbl = set()
            for c in chunks:
                kind, idx, o, s, K, half = c[0], c[1], c[2], c[3], c[4], c[5]
                if kind == "A" and o not in done_dbl:
                    emit_dbl_copy(o, s, nc.gpsimd if id(c) in gp_subs
                                  else nc.vector)
                    done_dbl.add(o)
                emit_subs(c)

            # squares per chunk per coord (ACT)
            for c in chunks:
                F = c[8]
                off_e = seg_off[id(c)] * D
                for cc in range(2):
                    nc.scalar.activation(
                        out=seg_ap(sqt[cc], off_e, F),
                        in_=seg_ap(subt[cc], off_e, F),
                        func=mybir.ActivationFunctionType.Square)

            # d2 adds: all-f16 contiguous tensor_tensor hits the HW 2x path
            for c in chunks:
                F = c[8]
                off_e = seg_off[id(c)] * D
                nc.vector.tensor_tensor(out=seg_ap(d2, off_e, F),
                                        in0=seg_ap(sqx, off_e, F),
                                        in1=seg_ap(sqy, off_e, F),
                                        op=mybir.AluOpType.add)

            # ---- min over dj (one fully-contiguous f16 reduce per group —
            # this shape measures ~0.59 ns/elem on HW), then min over di ----
            pdist = singles.tile([P, PP], DT_BULK)
            m1t = big.tile([P, NSEG], DT_BULK, tag="m1")

            d2f = d2[:]
            ed = d2f.ap[-1][0]
            m1f = m1t[:]
            em1 = m1f.ap[-1][0]

            def tree_min(u0, un):
                iv = bass.AP(tensor=d2f.tensor,
                             offset=d2f.offset + u0 * D * ed,
                             ap=[d2f.ap[0], [D * ed, un], [ed, D]])
                ov = bass.AP(tensor=m1f.tensor, offset=m1f.offset + u0 * em1,
                             ap=[m1f.ap[0], [em1, un]])
                nc.vector.tensor_reduce(out=ov, in_=iv,
                                        axis=mybir.AxisListType.X,
                                        op=mybir.AluOpType.min)

            def min2_chunk(c):
                kind, idx, o, s, K, half, qoff, qn, F = c
                u0 = seg_off[id(c)]
                if kind == "A":
                    iv = bass.AP(tensor=m1f.tensor,
                                 offset=m1f.offset + u0 * em1,
                                 ap=[m1f.ap[0], [D * K * em1, s], [em1, K],
                                     [K * em1, D]])
                else:
                    iv = bass.AP(tensor=m1f.tensor,
                                 offset=m1f.offset + u0 * em1,
                                 ap=[m1f.ap[0], [D * em1, half], [em1, D]])
                ov = pdist[:, qoff : qoff + qn]
                nc.vector.tensor_reduce(out=ov, in_=iv,
                                        axis=mybir.AxisListType.X,
                                        op=mybir.AluOpType.min)

            rqt = small.tile([P, PP], F16, tag="rqt")
            rrt = small.tile([P, PP], F16, tag="rrt")
            penn = small.tile([P, PP], F16, tag="penn")
            st_sb = small.tile([PP, NL], F16, tag="st_sb")

            wpos = singles.tile([P, NL], F16)
            mc2 = singles.tile([PP, B], F16)
            tchb = singles.tile([P, 1], F16, tag="tchb")

            staged = []

            def stage_pe_inputs():
                # ACT copies staging every PE operand so each PE op has
                # ACT-only deps (one sync wait); the DVE pre-touch of xtb
                # lets the later rq multiply read inv_pd^2 without a DMA
                # wait. Emitted late so the scheduler keeps them off the
                # critical stage-A window.
                nc.scalar.copy(out=wpos[:], in_=xtb[:, XO_W : XO_W + NL])
                nc.scalar.copy(out=mc2[:], in_=mct[:])
                nc.vector.tensor_copy(out=tchb[:], in_=xtb[:, 0:1])
                staged.append(True)

            for gi, g in enumerate(g_chunks):
                u0 = min(seg_off[id(c)] for c in g)
                u1 = max(seg_off[id(c)] + c[8] // D for c in g)
                un = u1 - u0
                if un % 2:
                    un += 1          # cover the pad unit; keeps 2x reduce
                tree_min(u0, un)
                for c in g:
                    min2_chunk(c)
                if not staged:
                    stage_pe_inputs()
                q0, q1 = g_qr[gi]
                # rq = d2min * inv_pd^2; rr = sqrt(rq) = dist/pd;
                # pen = relu(1 - rr)  (all per-pair, f16)
                nc.vector.tensor_mul(rqt[:, q0:q1], pdist[:, q0:q1],
                                     prc2[:, q0:q1])
                nc.scalar.activation(out=rrt[:, q0:q1], in_=rqt[:, q0:q1],
                                     func=mybir.ActivationFunctionType.Sqrt)
                nc.scalar.activation(out=penn[:, q0:q1], in_=rrt[:, q0:q1],
                                     func=mybir.ActivationFunctionType.Relu,
                                     bias=1.0, scale=-1.0)
            # ---- tail: S^T = pen^T @ w (pen stationary -> no transpose
            # needed), stage to SBUF, @M, mask, out ----
            ps2 = psum.tile([PP, NL], F32, tag="ps2")
            nc.tensor.matmul(ps2[:], penn[:], wpos[:], start=True, stop=True)
            nc.scalar.copy(out=st_sb[:], in_=ps2[:])
            ps3 = psum.tile([NL, B], F32, tag="ps3")
            nc.tensor.matmul(ps3[:], st_sb[:], mc2[:], start=True, stop=True)
            lout = small.tile([NL, B], F32, tag="lout")
            nc.vector.tensor_mul(lout[:], ps3[:], movt)
            nc.sync.dma_start(out=out[:], in_=lout[:])

    return nc


def _prepare(inputs):
    x = np.ascontiguousarray(inputs["x"], dtype=np.float32)
    extent = np.asarray(inputs["extent"], dtype=np.float32)
    wfa = np.asarray(inputs["world_from_agent"], dtype=np.float32)
    speed = np.asarray(inputs["curr_speed"], dtype=np.float32)
    scene = np.asarray(inputs["scene_index"])

    R = wfa[:, :2, :2]
    tr = wfa[:, :2, 2]
    yaw_off = np.arctan2(R[:, 1, 0], R[:, 0, 0]).astype(np.float32)
    agt_rad = extent[:, 1] / 2.0
    cent_min = -(extent[:, 0] / 2.0) + agt_rad
    cent_max = (extent[:, 0] / 2.0) - agt_rad
    lin = np.linspace(0.0, 1.0, D, dtype=np.float32)
    cent_x = (cent_min[:, None] + (cent_max - cent_min)[:, None] * lin).astype(
        np.float32)
    pd = (agt_rad[:, None] + agt_rad[None, :] + BUFFER_DIST).astype(np.float32)
    moving = (np.abs(speed) > SPEED_TH)

    # contiguous scene blocks (scene_index is sorted)
    _, starts, counts = np.unique(scene, return_index=True, return_counts=True)
    scenes = [(int(o), int(s)) for o, s in zip(starts, counts)]
    assert sum(s for _, s in scenes) == B
    for o, s in scenes:
        assert (scene[o : o + s] == scene[o]).all()

    pairs_i = []
    pairs_j = []
    for (o, s, K, half) in _rects(scenes):
        for i in range(s):
            for k in range(1, K + 1):
                pairs_i.append(o + i)
                pairs_j.append(o + (i + k) % s)
        for i in range(half):
            pairs_i.append(o + i)
            pairs_j.append(o + i + half)
    pairs_i = np.array(pairs_i, dtype=np.int64)
    pairs_j = np.array(pairs_j, dtype=np.int64)
    PP = len(pairs_i)
    inv_pd2 = (1.0 / pd[pairs_i, pairs_j] ** 2).astype(np.float32)

    # pair -> agent incidence matrix for the tail matmul
    mcon = np.zeros((PP, B), dtype=np.float16)
    mcon[np.arange(PP), pairs_i] = 1.0
    mcon[np.arange(PP), pairs_j] = 1.0

    twopi = 2.0 * np.pi
    consts = np.concatenate([
        np.concatenate([R[:, 0, 0], R[:, 1, 0],          # gA
                        R[:, 0, 1], R[:, 1, 1]]).astype(np.float16),
        np.concatenate([tr[:, 0], tr[:, 1]]).astype(np.float32)
          .view(np.float16),                             # gT (f32 island)
        np.concatenate([2.0 + yaw_off / twopi, 2.25 + yaw_off / twopi])
          .astype(np.float32).view(np.float16),          # shifts2 (f32)
        cent_x.reshape(-1).astype(np.float16),
    ])
    assert consts.shape[0] == XWA - 3 * B

    w = DECAY_RATE ** np.arange(T, dtype=np.float32)
    w = w / w.sum()
    wz = np.zeros((P, NL), dtype=np.float32)
    for nl in range(NL):
        wz[nl * T : (nl + 1) * T, nl] = w / B * WSCALE

    ident = np.zeros((P, NL), dtype=np.float32)
    for nl in range(NL):
        ident[nl, nl] = 1.0

    XWB = _xin_width_b(PP)
    xinb_row = np.empty((P, XWB), dtype=np.float32)
    xinb_row[:, XO_MVR : XO_MVR + B] = (moving.astype(np.float32)
                                        / WSCALE)[None, :]
    xinb_row[:, XO_W : XO_W + NL] = wz
    xinb_row[:, XO_ID : XO_ID + NL] = ident
    xinb_row[:, XO_PRC : XO_PRC + PP] = inv_pd2[None, :]
    xinb_row = xinb_row.astype(np.float16)

    in_maps = []
    for c in range(NCORES):
        xs = x[:, c * NL : (c + 1) * NL, :, :]          # (B, NL, T, 6)
        xs = xs[..., [0, 1, 3]]                          # (B, NL, T, 3)
        xdat = xs.transpose(1, 2, 3, 0).reshape(P, 3 * B).astype(np.float16)
        xina = np.empty((P, XWA), dtype=np.float16)
        xina[:, 0 : 3 * B] = xdat
        xina[:, 3 * B :] = consts[None, :]
        in_maps.append({"xina": xina, "xinb": xinb_row, "mcon": mcon})

    return scenes, PP, in_maps, moving


_CACHE = {}


def _get_nc(scenes, PP):
    key = (tuple(scenes), PP)
    if key not in _CACHE:
        _CACHE[key] = _build_nc(scenes, PP)
    return _CACHE[key]


def _run(inputs, trace=False):
    scenes, PP, in_maps, moving = _prepare(inputs)
    nc = _get_nc(scenes, PP)
    res = run_bass_kernel_spmd(nc, in_maps, core_ids=list(range(NCORES)),
                               trace=trace)
    outf = np.zeros((B, N), dtype=np.float32)
    for c in range(NCORES):
        lc = res.results[c]["loss"]                      # (NL, B)
        for nl in range(NL):
            outf[:, c * NL + nl] = lc[nl]
    return outf, res


def kernel(**inputs):
    outf, _ = _run(inputs, trace=False)
    return outf


def _ensure_ntff_hook():
    """Register the axon NTFF profile hook if the container's antenv lacks it."""
    try:
        from antenv.axon_hooks import get_axon_ntff_profile_hook  # noqa: F401
        return
    except ImportError:
        pass
    import types

    if "/root/.axon_site" not in sys.path:
        sys.path.insert(0, "/root/.axon_site")
    from trn_agent_boot.trn_boot import _ntff_profile_via_ctypes

    hook = _ntff_profile_via_ctypes("/opt/axon/libaxon_pjrt.so")
    mod = types.ModuleType("antenv.axon_hooks")
    mod.get_axon_ntff_profile_hook = lambda: hook
    mod.set_axon_ntff_profile_hook = lambda h: None
    sys.modules["antenv.axon_hooks"] = mod


def run_traced(inputs):
    """Correctness output + profiled exec time (ns) via NTFF trace."""
    _ensure_ntff_hook()
    outf, res = _run(inputs, trace=True)
    return outf, res.exec_time_ns


# revision 59
# speedup vs baseline: 1.2392x; 1.2392x over previous
"""AgentCollisionLoss Trainium2 kernel.

Full inputs -> full output. Shards the N (sample) dim across 8 NeuronCores
(2 samples per core), computes the pairwise agent-collision loss on device,
and gathers the per-core (NL, B) losses into the full (B, N) output.

Device layout (per core):
  partition p = n_local*T + t            (104 rows)
  Inputs:  xina [P, 640] f16 (x-data + replicated per-agent consts, with
           f32 islands bitcast into f16 column pairs) on the sync queue;
           xinb f16 + mcon f16 issued from the ACT queue.
  Stage A: world-frame disk centroids CXY [P, 2, B, D] f32, emitted
           per scene so the first rect's subs start early; sin/cos via the
           ACT Sin table (loaded before the data lands); the Sqrt table is
           pre-loaded by a dummy op ordered right after Sin.
  Stage B: per-rect outer-difference subs (DVE, f32->f16) -> squares
           (ACT) -> d2 adds (DVE, all-f16 contiguous = HW 2x path) into
           one packed f16 d2 tile, (pair, di, dj)-ordered.
  Min:     3-op tensor_tensor min tree over dj per pipeline group, then
           per-chunk strided reduces over di -> pdist [P, PP] f16.
  Pen:     rq = pdist * inv_pd^2 (DVE), rr = sqrt(rq) (ACT),
           pen = relu(1 - rr) (ACT, scale=-1 bias=1).
  Tail:    S^T[q,nl] = sum_p pen[p,q] w[p,nl] via one PE matmul with pen
           stationary (no transpose needed), stage to SBUF, then
           loss[nl,i] = sum_q S^T[q,nl] M[q,i] with the host-built
           pair->agent incidence matrix M (moving mask and 1/WSCALE
           folded into M's columns); copy PSUM->SBUF and DMA out.

Walrus on this build allows ONE embedded sync wait per instruction, so
every op is arranged to have a single new cross-engine dependency (the
pre-touch/staging-copy pattern); the Tile drain tail is patched to skip
the redundant barriers + sem clearing (the NRT NEFF epilogue re-zeroes
all declared semaphores on every engine anyway).
"""

import os
import sys

import numpy as np

for _p in ("/opt/trn_rl_repo", "/root/.axon_site/_ro/trn_rl_repo"):
    if os.path.isdir(_p) and _p not in sys.path:
        sys.path.insert(0, _p)

import bass_rust
import concourse.bass as bass
import concourse.mybir as mybir
import concourse.tile as tile
from concourse.bass_utils import run_bass_kernel_spmd
from concourse.vector_clock import ScopedClock


def _split_drain_and_barrier(self, tick_clock, wait_clock):
    """Kernel-tail drain, one semaphore per drain instruction.

    The walrus build in this container rejects instructions carrying more
    than one embedded sync wait ("Too many sync wait commands"). Tile's
    stock tail emits a single drain waiting on the full global clock, so
    split it: one drain per nonzero proc tick. add_sem_waits elides waits
    the engine has already observed, so each drain carries exactly one.
    """
    gc = list(tick_clock.global_clock)
    engs = [self.nc.sync, self.nc.vector, self.nc.scalar, self.nc.gpsimd,
            self.nc.tensor]
    nd = 0
    for idx, tick in enumerate(gc):
        if tick <= 0:
            continue
        v = [0] * len(gc)
        v[idx] = tick
        d = engs[nd % len(engs)].drain()
        nd += 1
        wait_clock.add_sem_waits(
            d.ins, ScopedClock({None: bass_rust.VectorClock(v)})
        )
    # No barriers and no sem_clear here: the NRT NEFF epilogue already
    # zeroes every declared semaphore on every engine and ends with its own
    # all-engine barrier, so Tile's tail would duplicate ~2-3us of work on
    # the critical path. The split drains above still flush the DMA queues
    # (the out DMA) before the epilogue runs. Re-execution stays correct
    # because the epilogue's sweep covers the Tile sems (verified by
    # running the NEFF twice and checking outputs).
    assert self.sems is not None
    popped = self.nc._tile_sem_poison_stack.pop()
    assert popped is self._sem_poison
    sems = list(self.sems.allocated().values())
    sem_nums = [getattr(s, "num", s) for s in sems]
    self.nc._state.prepend_free_semaphores(sem_nums)
    for poison_set in self.nc._tile_sem_poison_stack:
        poison_set.update(sem_nums)


tile.TileContext._drain_and_barrier = _split_drain_and_barrier

B, N, T, D = 32, 16, 52, 5
NCORES = 8
NL = N // NCORES          # samples per core
P = NL * T                # partition rows per core
BUFFER_DIST = 0.2
DECAY_RATE = 0.9
SPEED_TH = 0.5
WSCALE = 64.0             # f16-normal range scaling for the decay weights

F32 = mybir.dt.float32
F16 = mybir.dt.float16
PI = float(np.pi)

DT_BULK = F16


def _rects(scenes):
    """Circulant half-pair rects per scene, largest first.

    Each unordered same-scene pair {i, j} is covered exactly once:
    rect A: (i, k) for i in [0,s), k in [1,K], j = (i+k) mod s, K=(s-1)//2
    rect B (even s): (i, s/2) for i in [0, s/2), j = i + s/2
    Returns [(o, s, K, half)] with half = s//2 if s even else 0.
    """
    out = []
    for (o, s) in scenes:
        K = (s - 1) // 2
        half = s // 2 if s % 2 == 0 else 0
        out.append((o, s, K, half))
    out.sort(key=lambda r: -(r[1] * r[2] + r[3]))
    return out


# xinA (f16, replicated consts carry f32 islands as bitcast pairs):
# x0(B) x1(B) yaw(B) | gA(2B) gB(2B) | gT(2B f32 = 4B slots) |
# shifts2(2B f32 = 4B slots) | cent(B*D)
XO_GA = 3 * B
XO_GT = XO_GA + 4 * B          # f32-bitcast island
XO_SH = XO_GT + 4 * B          # f32-bitcast island
XO_CE = XO_SH + 4 * B
XWA = XO_CE + B * D
# xinB (f16): movt/WSCALE(B) | w*WSCALE(NL) | ident(NL) | inv_pd^2(PP)
XO_MVR = 0
XO_W = XO_MVR + B
XO_ID = XO_W + NL
XO_PRC = XO_ID + NL


def _xin_width_b(PP):
    return XO_PRC + PP


def _build_nc(scenes, PP):
    """Build the SPMD Bass program. `scenes` = [(offset, size)], PP = #pairs."""
    nc = bass.Bass()
    assert PP <= 128

    XWB = _xin_width_b(PP)
    xina = nc.dram_tensor("xina", [P, XWA], F16, kind="ExternalInput")
    xinb = nc.dram_tensor("xinb", [P, XWB], F16, kind="ExternalInput")
    mcon = nc.dram_tensor("mcon", [PP, B], F16, kind="ExternalInput")
    out = nc.dram_tensor("loss", [NL, B], F32, kind="ExternalOutput")

    rects = _rects(scenes)

    # chunk list: (kind, rect-idx, o, s, K, half, q-offset, q-count, F)
    # pair/q order: per rect (A-pairs (i,k) i-major, then its half pairs)
    chunks = []
    po = 0
    for idx, (o, s, K, half) in enumerate(rects):
        if K >= 1:
            chunks.append(("A", idx, o, s, K, half, po, s * K, s * K * D * D))
        if half:
            chunks.append(("H", idx, o, s, K, half, po + s * K, half,
                           half * D * D))
        po += s * K + half
    assert po == PP

    # d2 segment offsets (in (pair,di) units of 5 dj elems); group
    # boundaries padded to even unit counts (cheap, keeps ranges aligned)
    seg_off = {}
    pad_units = []
    so = 0
    prev_grp = None
    for c in chunks:
        g = 0 if c[1] <= 1 else 1
        if prev_grp is not None and g != prev_grp and so % 2:
            pad_units.append(so)
            so += 1
        prev_grp = g
        seg_off[id(c)] = so
        so += c[8] // D
    if so % 2:
        pad_units.append(so)
        so += 1
    NSEG = so

    # two pipeline groups: group 0 = the two largest rects (d2 adds on DVE),
    # group 1 = the rest (d2 adds on gpsimd). Groups are contiguous q ranges.
    grp_of = {}
    for c in chunks:
        grp_of[id(c)] = 0 if c[1] <= 1 else 1
    g_chunks = [[c for c in chunks if grp_of[id(c)] == g] for g in (0, 1)]
    g_chunks = [g for g in g_chunks if g]
    g_qr = []
    for g in g_chunks:
        q0 = min(c[6] for c in g)
        q1 = max(c[6] + c[7] for c in g)
        g_qr.append((q0, q1))

    # gpsimd measures 2.4-12 ns/elem on HW (vs DVE 0.59-1.1) — keep ALL
    # stage-B work on DVE; gpsimd stays idle.
    gp_subs = set()

    with tile.TileContext(nc) as tc:
        with (
            tc.tile_pool(name="singles", bufs=1) as singles,
            tc.tile_pool(name="small", bufs=1) as small,
            tc.tile_pool(name="big", bufs=1) as big,
            tc.tile_pool(name="psum", bufs=1, space="PSUM") as psum,
        ):
            # ---- loads: xina on sync, consts on vector, xinb+mcon on
            # tensor, so the three DIRECT2D descriptor-gen steps overlap ----
            xta = singles.tile([P, XWA], F16)
            nc.sync.dma_start(out=xta[:], in_=xina[:])
            # xinb + mcon issue from the ACT queue: the sync queue then has
            # only xina ahead of it (lands earliest), and ACT's early idle
            # window absorbs the ~0.7us descriptor-gen per DMA.
            xtb = singles.tile([P, XWB], F16)
            nc.scalar.dma_start(out=xtb[:], in_=xinb[:])
            mct = singles.tile([PP, B], F16)
            nc.scalar.dma_start(out=mct[:], in_=mcon[:])

            # (no xta pre-touch: u2, the first DVE op, reads only xta and
            # thus carries the xina DMA-queue wait itself; later DVE
            # readers of xta elide it)
            gA = xta[:, XO_GA : XO_GA + 2 * B]
            gT = xta[:, XO_GT : XO_GT + 4 * B].bitcast(F32)
            shifts2 = xta[:, XO_SH : XO_SH + 4 * B].bitcast(F32)
            cxc = xta[:, XO_CE : XO_CE + B * D]
            x0 = xta[:, 0:B]
            yw = xta[:, 2 * B : 3 * B]
            prc2 = xtb[:, XO_PRC : XO_PRC + PP]     # inv_pd^2 per pair

            def rep2(apx, w):
                """view [P, 2, w] reading apx's first w elems twice"""
                return bass.AP(tensor=apx.tensor, offset=apx.offset,
                               ap=[apx.ap[0], [0, 2], [1, w]])

            # ---- stage A ----
            # u = yaw/2pi + (shift + yoff/2pi)   (shift 2.0 -> sin, 2.25 -> cos)
            u2 = small.tile([P, 2, B], F32)
            nc.vector.scalar_tensor_tensor(
                out=u2[:], in0=rep2(yw, B), scalar=1.0 / (2.0 * PI),
                in1=shifts2.rearrange("p (c i) -> p c i", c=2),
                op0=mybir.AluOpType.mult, op1=mybir.AluOpType.add)
            # round-to-nearest-even via the 1.5*2^23 magic constant
            MAGIC = 12582912.0
            kf = small.tile([P, 2, B], F32)
            nc.vector.tensor_scalar(
                out=kf[:], in0=u2[:], scalar1=MAGIC, scalar2=MAGIC,
                op0=mybir.AluOpType.add, op1=mybir.AluOpType.subtract)
            fr = small.tile([P, 2, B], F32)
            nc.vector.tensor_sub(fr[:], u2[:], kf[:])
            # sincos[:, 0:32] = sin(yawg), [:, 32:64] = cos(yawg)
            sincos = small.tile([P, 2 * B], F32)
            nc.scalar.activation(out=sincos[:].rearrange("p (c i) -> p c i", c=2),
                                 in_=fr[:],
                                 func=mybir.ActivationFunctionType.Sin,
                                 bias=0.0, scale=2.0 * PI)
            # dummy sqrt so the Sqrt table load happens right after sin in
            # the idle-ACT window instead of on the critical tail. Reading
            # sincos (not ones) keeps the scheduler from hoisting it (and
            # its table load) in front of sin.
            dum = small.tile([P, 1], F32, tag="dum")
            nc.scalar.activation(out=dum[:], in_=sincos[:, 0:1],
                                 func=mybir.ActivationFunctionType.Sqrt)

            # pos_g for both coords: pg[p, c, i], c=0 -> x, 1 -> y
            # m12[p, xsel, c, i] = x_xsel * g_{xsel,c}  in one multiply
            m12 = small.tile([P, 2, 2, B], F32)
            xx = bass.AP(tensor=xta.tensor, offset=x0.offset,
                         ap=[x0.ap[0], [B, 2], [0, 2], [1, B]])
            gAB = bass.AP(tensor=xta.tensor, offset=gA.offset,
                          ap=[gA.ap[0], [2 * B, 2], [B, 2], [1, B]])
            nc.vector.tensor_mul(m12[:], xx, gAB)
            pg = small.tile([P, 2, B], F32)
            nc.vector.tensor_add(pg[:], m12[:, 0], m12[:, 1])
            nc.vector.tensor_add(pg[:], pg[:],
                                 gT.rearrange("p (c i) -> p c i", c=2))

            # CXY[p, c, i, di] = cent_x(i,di) * cs(c,i) + pg(c,i), emitted
            # per scene (inside the sub loop) so the first rect's subs
            # start before the other scenes' cxy is computed
            cxy = singles.tile([P, 2, B, D], F32)

            def emit_cxy(o, s):
                cs_sel = bass.AP(tensor=sincos.tensor,
                                 offset=sincos[:].offset + B + o,
                                 ap=[sincos[:].ap[0], [-B, 2], [1, s], [0, D]])
                cx_rep = bass.AP(tensor=xta.tensor, offset=cxc.offset + o * D,
                                 ap=[cxc.ap[0], [0, 2], [D, s], [1, D]])
                pg_bc = bass.AP(tensor=pg.tensor, offset=pg[:].offset + o,
                                ap=[pg[:].ap[0], [B, 2], [1, s], [0, D]])
                cxy_sl = bass.AP(tensor=cxy.tensor,
                                 offset=cxy[:].offset + o * D,
                                 ap=[cxy[:].ap[0], [B * D, 2], [D, s], [1, D]])
                nc.vector.tensor_mul(cxy_sl, cx_rep, cs_sel)
                nc.vector.tensor_add(cxy_sl, cxy_sl, pg_bc)

            cxyf = cxy[:].rearrange("p c i d -> p (c i d)")
            pap = cxyf.ap[0]
            e = cxyf.ap[-1][0]
            NPTS = B * D

            # doubled per-scene point lists (wrap j = (i+k) mod s becomes a
            # linear read). Kept on DVE: subs read cxy AND cxy2 and may
            # carry only one sync wait, so both must be DVE-written.
            DBL = 2 * NPTS
            cxy2 = singles.tile([P, 2, DBL], F32)
            c2f = cxy2[:].rearrange("p c d -> p (c d)")
            pap2 = c2f.ap[0]
            e2 = c2f.ap[-1][0]
            dbl_off = {}
            do_ = 0
            for (o, s) in scenes:
                dbl_off[o] = do_
                do_ += 2 * D * s

            subx = big.tile([P, NSEG * D], DT_BULK, tag="subx")
            suby = big.tile([P, NSEG * D], DT_BULK, tag="suby")
            sqx = big.tile([P, NSEG * D], DT_BULK, tag="sqx")
            sqy = big.tile([P, NSEG * D], DT_BULK, tag="sqy")
            d2 = big.tile([P, NSEG * D], DT_BULK, tag="d2")
            subt = {0: subx, 1: suby}
            sqt = {0: sqx, 1: sqy}

            def seg_ap(t, off_e, F):
                tf = t[:]
                es = tf.ap[-1][0]
                return bass.AP(tensor=tf.tensor, offset=tf.offset + off_e * es,
                               ap=[tf.ap[0], [es, F]])

            def emit_dbl_copy(o, s, eng):
                # each scene's doubled block is built by the engine that
                # subs it, so the subs see a single-engine cxy/cxy2 pair
                # (DVE) or carry one DVE wait (gpsimd)
                in_ap = bass.AP(tensor=cxyf.tensor,
                                offset=cxyf.offset + o * D * e,
                                ap=[pap, [NPTS * e, 2], [0, 2], [e, D * s]])
                out_ap = bass.AP(tensor=c2f.tensor,
                                 offset=c2f.offset + dbl_off[o] * e2,
                                 ap=[pap2, [DBL * e2, 2], [D * s * e2, 2],
                                     [e2, D * s]])
                eng.tensor_copy(out=out_ap, in_=in_ap)

            def emit_subs(c):
                kind, idx, o, s, K, half, qoff, qn, F = c
                off_e = seg_off[id(c)] * D
                eng = nc.gpsimd if id(c) in gp_subs else nc.vector
                if kind == "A":
                    w = D * K
                    for cc in range(2):
                        a_ap = bass.AP(
                            tensor=cxyf.tensor,
                            offset=cxyf.offset + (cc * NPTS + o * D) * e,
                            ap=[pap, [D * e, s], [e, D], [0, w]])
                        b_ap = bass.AP(
                            tensor=c2f.tensor,
                            offset=c2f.offset + (cc * DBL + dbl_off[o] + D) * e2,
                            ap=[pap2, [D * e2, s], [0, D], [e2, w]])
                        eng.tensor_tensor(
                            out=seg_ap(subt[cc], off_e, F), in0=a_ap,
                            in1=b_ap, op=mybir.AluOpType.subtract)
                else:
                    for cc in range(2):
                        a_ap = bass.AP(
                            tensor=cxyf.tensor,
                            offset=cxyf.offset + (cc * NPTS + o * D) * e,
                            ap=[pap, [D * e, half], [e, D], [0, D]])
                        b_ap = bass.AP(
                            tensor=cxyf.tensor,
                            offset=cxyf.offset + (cc * NPTS + (o + half) * D) * e,
                            ap=[pap, [D * e, half], [0, D], [e, D]])
                        eng.tensor_tensor(
                            out=seg_ap(subt[cc], off_e, F), in0=a_ap,
                            in1=b_ap, op=mybir.AluOpType.subtract)

            # interleave the per-scene cxy, the cxy2 doubling copies, and
            # the subs, in rect (largest-first) order
            done_cxy = set()
            for c in chunks:
                kind, idx, o, s, K, half = c[0], c[1], c[2], c[3], c[4], c[5]
                if o not in done_cxy:
                    emit_cxy(o, s)
                    if kind == "A":
                        emit_dbl_copy(o, s, nc.vector)
                    done_cxy.add(o)
                emit_subs(c)

            # squares per chunk per coord (ACT)
            for c in chunks:
                F = c[8]
                off_e = seg_off[id(c)] * D
                for cc in range(2):
                    nc.scalar.activation(
                        out=seg_ap(sqt[cc], off_e, F),
                        in_=seg_ap(subt[cc], off_e, F),
                        func=mybir.ActivationFunctionType.Square)

            # d2 adds: all-f16 contiguous tensor_tensor hits the HW 2x path
            for c in chunks:
                F = c[8]
                off_e = seg_off[id(c)] * D
                nc.vector.tensor_tensor(out=seg_ap(d2, off_e, F),
                                        in0=seg_ap(sqx, off_e, F),
                                        in1=seg_ap(sqy, off_e, F),
                                        op=mybir.AluOpType.add)

            # ---- min over dj (3-op tt-min tree per group: tensor_reduce
            # never hits the HW fast path, the tree reads fewer elems),
            # then min over di ----
            pdist = singles.tile([P, PP], DT_BULK)
            m2t = big.tile([P, NSEG, 2], DT_BULK, tag="m2")
            m1t = big.tile([P, NSEG], DT_BULK, tag="m1")

            d2f = d2[:]
            ed = d2f.ap[-1][0]
            m2f = m2t[:].rearrange("p a b -> p (a b)")
            em2 = m2f.ap[-1][0]
            m1f = m1t[:]
            em1 = m1f.ap[-1][0]

            def tree_min(u0, un):
                n = un
                in0 = bass.AP(tensor=d2f.tensor,
                              offset=d2f.offset + u0 * D * ed,
                              ap=[d2f.ap[0], [D * ed, n], [ed, 2]])
                in1 = bass.AP(tensor=d2f.tensor,
                              offset=d2f.offset + (u0 * D + 2) * ed,
                              ap=[d2f.ap[0], [D * ed, n], [ed, 2]])
                o2 = bass.AP(tensor=m2f.tensor,
                             offset=m2f.offset + u0 * 2 * em2,
                             ap=[m2f.ap[0], [2 * em2, n], [em2, 2]])
                nc.vector.tensor_tensor(out=o2, in0=in0, in1=in1,
                                        op=mybir.AluOpType.min)
                ia = bass.AP(tensor=m2f.tensor,
                             offset=m2f.offset + u0 * 2 * em2,
                             ap=[m2f.ap[0], [2 * em2, n]])
                ib = bass.AP(tensor=m2f.tensor,
                             offset=m2f.offset + (u0 * 2 + 1) * em2,
                             ap=[m2f.ap[0], [2 * em2, n]])
                o1 = bass.AP(tensor=m1f.tensor, offset=m1f.offset + u0 * em1,
                             ap=[m1f.ap[0], [em1, n]])
                nc.vector.tensor_tensor(out=o1, in0=ia, in1=ib,
                                        op=mybir.AluOpType.min)
                ic = bass.AP(tensor=d2f.tensor,
                             offset=d2f.offset + (u0 * D + 4) * ed,
                             ap=[d2f.ap[0], [D * ed, n]])
                nc.vector.tensor_tensor(out=o1, in0=o1, in1=ic,
                                        op=mybir.AluOpType.min)

            def min2_chunk(c):
                kind, idx, o, s, K, half, qoff, qn, F = c
                u0 = seg_off[id(c)]
                if kind == "A":
                    iv = bass.AP(tensor=m1f.tensor,
                                 offset=m1f.offset + u0 * em1,
                                 ap=[m1f.ap[0], [D * K * em1, s], [em1, K],
                                     [K * em1, D]])
                else:
                    iv = bass.AP(tensor=m1f.tensor,
                                 offset=m1f.offset + u0 * em1,
                                 ap=[m1f.ap[0], [D * em1, half], [em1, D]])
                ov = pdist[:, qoff : qoff + qn]
                nc.vector.tensor_reduce(out=ov, in_=iv,
                                        axis=mybir.AxisListType.X,
                                        op=mybir.AluOpType.min)

            rqt = small.tile([P, PP], F16, tag="rqt")
            rrt = small.tile([P, PP], F16, tag="rrt")
            penn = small.tile([P, PP], F16, tag="penn")
            st_sb = small.tile([PP, NL], F16, tag="st_sb")

            wpos = singles.tile([P, NL], F16)
            mc2 = singles.tile([PP, B], F16)
            tchb = singles.tile([P, 1], F16, tag="tchb")

            staged = []

            def stage_pe_inputs():
                # ACT copies staging every PE operand so each PE op has
                # ACT-only deps (one sync wait); the gpsimd pre-touch of
                # xtb lets the rq multiply (gpsimd) read inv_pd^2 without
                # a DMA wait. Emitted late so the scheduler keeps them off
                # the critical stage-A window.
                nc.scalar.copy(out=wpos[:], in_=xtb[:, XO_W : XO_W + NL])
                nc.scalar.copy(out=mc2[:], in_=mct[:])
                nc.vector.tensor_copy(out=tchb[:], in_=xtb[:, 0:1])
                staged.append(True)

            for gi, g in enumerate(g_chunks):
                u0 = min(seg_off[id(c)] for c in g)
                u1 = max(seg_off[id(c)] + c[8] // D for c in g)
                tree_min(u0, u1 - u0)
                for c in g:
                    min2_chunk(c)
                if not staged:
                    stage_pe_inputs()
                q0, q1 = g_qr[gi]
                # rq = d2min * inv_pd^2; rr = sqrt(rq) = dist/pd;
                # pen = relu(1 - rr)  (all per-pair, f16). rq stays on
                nc.vector.tensor_mul(rqt[:, q0:q1], pdist[:, q0:q1],
                                     prc2[:, q0:q1])
                nc.scalar.activation(out=rrt[:, q0:q1], in_=rqt[:, q0:q1],
                                     func=mybir.ActivationFunctionType.Sqrt)
                nc.scalar.activation(out=penn[:, q0:q1], in_=rrt[:, q0:q1],
                                     func=mybir.ActivationFunctionType.Relu,
                                     bias=1.0, scale=-1.0)
            # ---- tail: S^T = pen^T @ w (pen stationary -> no transpose
            # needed), stage to SBUF, @M (mask+scale folded into M), out.
            # The out DMA issues from the DVE queue right after lout so no
            # cross-engine sem hop sits before the descriptor generation.
            ps2 = psum.tile([PP, NL], F32, tag="ps2")
            nc.tensor.matmul(ps2[:], penn[:], wpos[:], start=True, stop=True)
            nc.vector.tensor_copy(out=st_sb[:], in_=ps2[:])
            ps3 = psum.tile([NL, B], F32, tag="ps3")
            nc.tensor.matmul(ps3[:], st_sb[:], mc2[:], start=True, stop=True)
            lout = small.tile([NL, B], F32, tag="lout")
            nc.vector.tensor_copy(out=lout[:], in_=ps3[:])
            nc.sync.dma_start(out=out[:], in_=lout[:])

    return nc


def _prepare(inputs):
    x = np.ascontiguousarray(inputs["x"], dtype=np.float32)
    extent = np.asarray(inputs["extent"], dtype=np.float32)
    wfa = np.asarray(inputs["world_from_agent"], dtype=np.float32)
    speed = np.asarray(inputs["curr_speed"], dtype=np.float32)
    scene = np.asarray(inputs["scene_index"])

    R = wfa[:, :2, :2]
    tr = wfa[:, :2, 2]
    yaw_off = np.arctan2(R[:, 1, 0], R[:, 0, 0]).astype(np.float32)
    agt_rad = extent[:, 1] / 2.0
    cent_min = -(extent[:, 0] / 2.0) + agt_rad
    cent_max = (extent[:, 0] / 2.0) - agt_rad
    lin = np.linspace(0.0, 1.0, D, dtype=np.float32)
    cent_x = (cent_min[:, None] + (cent_max - cent_min)[:, None] * lin).astype(
        np.float32)
    pd = (agt_rad[:, None] + agt_rad[None, :] + BUFFER_DIST).astype(np.float32)
    moving = (np.abs(speed) > SPEED_TH)

    # contiguous scene blocks (scene_index is sorted)
    _, starts, counts = np.unique(scene, return_index=True, return_counts=True)
    scenes = [(int(o), int(s)) for o, s in zip(starts, counts)]
    assert sum(s for _, s in scenes) == B
    for o, s in scenes:
        assert (scene[o : o + s] == scene[o]).all()

    pairs_i = []
    pairs_j = []
    for (o, s, K, half) in _rects(scenes):
        for i in range(s):
            for k in range(1, K + 1):
                pairs_i.append(o + i)
                pairs_j.append(o + (i + k) % s)
        for i in range(half):
            pairs_i.append(o + i)
            pairs_j.append(o + i + half)
    pairs_i = np.array(pairs_i, dtype=np.int64)
    pairs_j = np.array(pairs_j, dtype=np.int64)
    PP = len(pairs_i)
    inv_pd2 = (1.0 / pd[pairs_i, pairs_j] ** 2).astype(np.float32)

    # pair -> agent incidence matrix for the tail matmul, with the moving
    # mask and the 1/WSCALE de-scaling folded into its columns
    mcol = (moving.astype(np.float32) / WSCALE)
    mcon = np.zeros((PP, B), dtype=np.float32)
    mcon[np.arange(PP), pairs_i] = mcol[pairs_i]
    mcon[np.arange(PP), pairs_j] = mcol[pairs_j]
    mcon = mcon.astype(np.float16)

    twopi = 2.0 * np.pi
    consts = np.concatenate([
        np.concatenate([R[:, 0, 0], R[:, 1, 0],          # gA
                        R[:, 0, 1], R[:, 1, 1]]).astype(np.float16),
        np.concatenate([tr[:, 0], tr[:, 1]]).astype(np.float32)
          .view(np.float16),                             # gT (f32 island)
        np.concatenate([2.0 + yaw_off / twopi, 2.25 + yaw_off / twopi])
          .astype(np.float32).view(np.float16),          # shifts2 (f32)
        cent_x.reshape(-1).astype(np.float16),
    ])
    assert consts.shape[0] == XWA - 3 * B

    w = DECAY_RATE ** np.arange(T, dtype=np.float32)
    w = w / w.sum()
    wz = np.zeros((P, NL), dtype=np.float32)
    for nl in range(NL):
        wz[nl * T : (nl + 1) * T, nl] = w / B * WSCALE

    ident = np.zeros((P, NL), dtype=np.float32)
    for nl in range(NL):
        ident[nl, nl] = 1.0

    XWB = _xin_width_b(PP)
    xinb_row = np.empty((P, XWB), dtype=np.float32)
    xinb_row[:, XO_MVR : XO_MVR + B] = (moving.astype(np.float32)
                                        / WSCALE)[None, :]
    xinb_row[:, XO_W : XO_W + NL] = wz
    xinb_row[:, XO_ID : XO_ID + NL] = ident
    xinb_row[:, XO_PRC : XO_PRC + PP] = inv_pd2[None, :]
    xinb_row = xinb_row.astype(np.float16)

    in_maps = []
    for c in range(NCORES):
        xs = x[:, c * NL : (c + 1) * NL, :, :]          # (B, NL, T, 6)
        xs = xs[..., [0, 1, 3]]                          # (B, NL, T, 3)
        xdat = xs.transpose(1, 2, 3, 0).reshape(P, 3 * B).astype(np.float16)
        xina = np.empty((P, XWA), dtype=np.float16)
        xina[:, 0 : 3 * B] = xdat
        xina[:, 3 * B :] = consts[None, :]
        in_maps.append({"xina": xina, "xinb": xinb_row, "mcon": mcon})

    return scenes, PP, in_maps, moving


_CACHE = {}


def _get_nc(scenes, PP):
    key = (tuple(scenes), PP)
    if key not in _CACHE:
        _CACHE[key] = _build_nc(scenes, PP)
    return _CACHE[key]


def _run(inputs, trace=False):
    scenes, PP, in_maps, moving = _prepare(inputs)
    nc = _get_nc(scenes, PP)
    res = run_bass_kernel_spmd(nc, in_maps, core_ids=list(range(NCORES)),
                               trace=trace)
    outf = np.zeros((B, N), dtype=np.float32)
    for c in range(NCORES):
        lc = res.results[c]["loss"]                      # (NL, B)
        for nl in range(NL):
            outf[:, c * NL + nl] = lc[nl]
    return outf, res


def kernel(**inputs):
    outf, _ = _run(inputs, trace=False)
    return outf


def _ensure_ntff_hook():
    """Register the axon NTFF profile hook if the container's antenv lacks it."""
    try:
        from antenv.axon_hooks import get_axon_ntff_profile_hook  # noqa: F401
        return
    except ImportError:
        pass
    import types

    if "/root/.axon_site" not in sys.path:
        sys.path.insert(0, "/root/.axon_site")
    from trn_agent_boot.trn_boot import _ntff_profile_via_ctypes

    hook = _ntff_profile_via_ctypes("/opt/axon/libaxon_pjrt.so")
    mod = types.ModuleType("antenv.axon_hooks")
    mod.get_axon_ntff_profile_hook = lambda: hook
    mod.set_axon_ntff_profile_hook = lambda h: None
    sys.modules["antenv.axon_hooks"] = mod


def run_traced(inputs):
    """Correctness output + profiled exec time (ns) via NTFF trace."""
    _ensure_ntff_hook()
    outf, res = _run(inputs, trace=True)
    return outf, res.exec_time_ns


# revision 60
# speedup vs baseline: 1.2643x; 1.0203x over previous
"""AgentCollisionLoss Trainium2 kernel.

Full inputs -> full output. Shards the N (sample) dim across 8 NeuronCores
(2 samples per core), computes the pairwise agent-collision loss on device,
and gathers the per-core (NL, B) losses into the full (B, N) output.

Device layout (per core):
  partition p = n_local*T + t            (104 rows)
  Inputs:  xina [P, 640] f16 (x-data + replicated per-agent consts, with
           f32 islands bitcast into f16 column pairs) on the sync queue;
           xinb f16 + mcon f16 issued from the ACT queue.
  Stage A: world-frame disk centroids CXY [P, 2, B, D] f32, emitted
           per scene so the first rect's subs start early; sin/cos via the
           ACT Sin table (loaded before the data lands); the Sqrt table is
           pre-loaded by a dummy op ordered right after Sin.
  Stage B: per-rect outer-difference subs (DVE, f32->f16) -> squares
           (ACT) -> d2 adds (DVE, all-f16 contiguous = HW 2x path) into
           one packed f16 d2 tile, (pair, di, dj)-ordered.
  Min:     3-op tensor_tensor min tree over dj per pipeline group, then
           per-chunk strided reduces over di -> pdist [P, PP] f16.
  Pen:     rq = pdist * inv_pd^2 (DVE), rr = sqrt(rq) (ACT),
           pen = relu(1 - rr) (ACT, scale=-1 bias=1).
  Tail:    S^T[q,nl] = sum_p pen[p,q] w[p,nl] via one PE matmul with pen
           stationary (no transpose needed), stage to SBUF, then
           loss[nl,i] = sum_q S^T[q,nl] M[q,i] with the host-built
           pair->agent incidence matrix M (moving mask and 1/WSCALE
           folded into M's columns); copy PSUM->SBUF and DMA out.

Walrus on this build allows ONE embedded sync wait per instruction, so
every op is arranged to have a single new cross-engine dependency (the
pre-touch/staging-copy pattern); the Tile drain tail is patched to skip
the redundant barriers + sem clearing (the NRT NEFF epilogue re-zeroes
all declared semaphores on every engine anyway).
"""

import os
import sys

import numpy as np

for _p in ("/opt/trn_rl_repo", "/root/.axon_site/_ro/trn_rl_repo"):
    if os.path.isdir(_p) and _p not in sys.path:
        sys.path.insert(0, _p)

import bass_rust
import concourse.bass as bass
import concourse.mybir as mybir
import concourse.tile as tile
from concourse.bass_utils import run_bass_kernel_spmd
from concourse.vector_clock import ScopedClock


def _split_drain_and_barrier(self, tick_clock, wait_clock):
    """Kernel-tail drain, one semaphore per drain instruction.

    The walrus build in this container rejects instructions carrying more
    than one embedded sync wait ("Too many sync wait commands"). Tile's
    stock tail emits a single drain waiting on the full global clock, so
    split it: one drain per nonzero proc tick. add_sem_waits elides waits
    the engine has already observed, so each drain carries exactly one.
    """
    gc = list(tick_clock.global_clock)
    engs = [self.nc.sync, self.nc.vector, self.nc.scalar, self.nc.gpsimd,
            self.nc.tensor]
    nd = 0
    for idx, tick in enumerate(gc):
        if tick <= 0:
            continue
        if idx >= len(engs):
            # DMA-queue procs: skip the drain. The engine that would carry
            # it otherwise stalls its NEFF-epilogue sem sweep on the out
            # DMA's completion sem (~+2us). The sweep itself (>=64 sems x
            # ~52ns serial, plus the final barrier) always outlasts the
            # DMA tail (~1.1us issue-to-land), so the NEFF cannot end
            # before the output lands; verified by repeated-run output
            # checks.
            continue
        v = [0] * len(gc)
        v[idx] = tick
        d = engs[nd % len(engs)].drain()
        nd += 1
        wait_clock.add_sem_waits(
            d.ins, ScopedClock({None: bass_rust.VectorClock(v)})
        )
    # No barriers and no sem_clear here: the NRT NEFF epilogue already
    # zeroes every declared semaphore on every engine and ends with its own
    # all-engine barrier, so Tile's tail would duplicate ~2-3us of work on
    # the critical path. The split drains above still flush the DMA queues
    # (the out DMA) before the epilogue runs. Re-execution stays correct
    # because the epilogue's sweep covers the Tile sems (verified by
    # running the NEFF twice and checking outputs).
    assert self.sems is not None
    popped = self.nc._tile_sem_poison_stack.pop()
    assert popped is self._sem_poison
    sems = list(self.sems.allocated().values())
    sem_nums = [getattr(s, "num", s) for s in sems]
    self.nc._state.prepend_free_semaphores(sem_nums)
    for poison_set in self.nc._tile_sem_poison_stack:
        poison_set.update(sem_nums)


tile.TileContext._drain_and_barrier = _split_drain_and_barrier

B, N, T, D = 32, 16, 52, 5
NCORES = 8
NL = N // NCORES          # samples per core
P = NL * T                # partition rows per core
BUFFER_DIST = 0.2
DECAY_RATE = 0.9
SPEED_TH = 0.5
WSCALE = 64.0             # f16-normal range scaling for the decay weights

F32 = mybir.dt.float32
F16 = mybir.dt.float16
PI = float(np.pi)

DT_BULK = F16


def _rects(scenes):
    """Circulant half-pair rects per scene, largest first.

    Each unordered same-scene pair {i, j} is covered exactly once:
    rect A: (i, k) for i in [0,s), k in [1,K], j = (i+k) mod s, K=(s-1)//2
    rect B (even s): (i, s/2) for i in [0, s/2), j = i + s/2
    Returns [(o, s, K, half)] with half = s//2 if s even else 0.
    """
    out = []
    for (o, s) in scenes:
        K = (s - 1) // 2
        half = s // 2 if s % 2 == 0 else 0
        out.append((o, s, K, half))
    out.sort(key=lambda r: -(r[1] * r[2] + r[3]))
    return out


# xinA (f16, replicated consts carry f32 islands as bitcast pairs):
# x0(B) x1(B) yaw(B) | gA(2B) gB(2B) | gT(2B f32 = 4B slots) |
# shifts2(2B f32 = 4B slots) | cent(B*D)
XO_GA = 3 * B
XO_GT = XO_GA + 4 * B          # f32-bitcast island
XO_SH = XO_GT + 4 * B          # f32-bitcast island
XO_CE = XO_SH + 4 * B
XWA = XO_CE + B * D
# xinB (f16): movt/WSCALE(B) | w*WSCALE(NL) | ident(NL) | inv_pd^2(PP)
XO_MVR = 0
XO_W = XO_MVR + B
XO_ID = XO_W + NL
XO_PRC = XO_ID + NL


def _xin_width_b(PP):
    return XO_PRC + PP


def _build_nc(scenes, PP):
    """Build the SPMD Bass program. `scenes` = [(offset, size)], PP = #pairs."""
    nc = bass.Bass()
    assert PP <= 128

    XWB = _xin_width_b(PP)
    xina = nc.dram_tensor("xina", [P, XWA], F16, kind="ExternalInput")
    xinb = nc.dram_tensor("xinb", [P, XWB], F16, kind="ExternalInput")
    mcon = nc.dram_tensor("mcon", [PP, B], F16, kind="ExternalInput")
    out = nc.dram_tensor("loss", [NL, B], F32, kind="ExternalOutput")

    rects = _rects(scenes)

    # chunk list: (kind, rect-idx, o, s, K, half, q-offset, q-count, F)
    # pair/q order: per rect (A-pairs (i,k) i-major, then its half pairs)
    chunks = []
    po = 0
    for idx, (o, s, K, half) in enumerate(rects):
        if K >= 1:
            chunks.append(("A", idx, o, s, K, half, po, s * K, s * K * D * D))
        if half:
            chunks.append(("H", idx, o, s, K, half, po + s * K, half,
                           half * D * D))
        po += s * K + half
    assert po == PP

    # d2 segment offsets (in (pair,di) units of 5 dj elems); group
    # boundaries padded to even unit counts (cheap, keeps ranges aligned)
    seg_off = {}
    pad_units = []
    so = 0
    prev_grp = None
    for c in chunks:
        g = 0 if c[1] <= 1 else 1
        if prev_grp is not None and g != prev_grp and so % 2:
            pad_units.append(so)
            so += 1
        prev_grp = g
        seg_off[id(c)] = so
        so += c[8] // D
    if so % 2:
        pad_units.append(so)
        so += 1
    NSEG = so

    # two pipeline groups: group 0 = the two largest rects (d2 adds on DVE),
    # group 1 = the rest (d2 adds on gpsimd). Groups are contiguous q ranges.
    grp_of = {}
    for c in chunks:
        grp_of[id(c)] = 0 if c[1] <= 1 else 1
    g_chunks = [[c for c in chunks if grp_of[id(c)] == g] for g in (0, 1)]
    g_chunks = [g for g in g_chunks if g]
    g_qr = []
    for g in g_chunks:
        q0 = min(c[6] for c in g)
        q1 = max(c[6] + c[7] for c in g)
        g_qr.append((q0, q1))

    # gpsimd measures 2.4-12 ns/elem on HW (vs DVE 0.59-1.1) — keep ALL
    # stage-B work on DVE; gpsimd stays idle.
    gp_subs = set()

    with tile.TileContext(nc) as tc:
        with (
            tc.tile_pool(name="singles", bufs=1) as singles,
            tc.tile_pool(name="small", bufs=1) as small,
            tc.tile_pool(name="big", bufs=1) as big,
            tc.tile_pool(name="psum", bufs=1, space="PSUM") as psum,
        ):
            # ---- loads: xina on sync, consts on vector, xinb+mcon on
            # tensor, so the three DIRECT2D descriptor-gen steps overlap ----
            xta = singles.tile([P, XWA], F16)
            nc.sync.dma_start(out=xta[:], in_=xina[:])
            # xinb + mcon issue from the ACT queue: the sync queue then has
            # only xina ahead of it (lands earliest), and ACT's early idle
            # window absorbs the ~0.7us descriptor-gen per DMA.
            xtb = singles.tile([P, XWB], F16)
            nc.scalar.dma_start(out=xtb[:], in_=xinb[:])
            mct = singles.tile([PP, B], F16)
            nc.scalar.dma_start(out=mct[:], in_=mcon[:])

            # (no xta pre-touch: u2, the first DVE op, reads only xta and
            # thus carries the xina DMA-queue wait itself; later DVE
            # readers of xta elide it)
            gA = xta[:, XO_GA : XO_GA + 2 * B]
            gT = xta[:, XO_GT : XO_GT + 4 * B].bitcast(F32)
            shifts2 = xta[:, XO_SH : XO_SH + 4 * B].bitcast(F32)
            cxc = xta[:, XO_CE : XO_CE + B * D]
            x0 = xta[:, 0:B]
            yw = xta[:, 2 * B : 3 * B]
            prc2 = xtb[:, XO_PRC : XO_PRC + PP]     # inv_pd^2 per pair

            def rep2(apx, w):
                """view [P, 2, w] reading apx's first w elems twice"""
                return bass.AP(tensor=apx.tensor, offset=apx.offset,
                               ap=[apx.ap[0], [0, 2], [1, w]])

            # ---- stage A ----
            # u = yaw/2pi + (shift + yoff/2pi)   (shift 2.0 -> sin, 2.25 -> cos)
            u2 = small.tile([P, 2, B], F32)
            nc.vector.scalar_tensor_tensor(
                out=u2[:], in0=rep2(yw, B), scalar=1.0 / (2.0 * PI),
                in1=shifts2.rearrange("p (c i) -> p c i", c=2),
                op0=mybir.AluOpType.mult, op1=mybir.AluOpType.add)
            # round-to-nearest-even via the 1.5*2^23 magic constant
            MAGIC = 12582912.0
            kf = small.tile([P, 2, B], F32)
            nc.vector.tensor_scalar(
                out=kf[:], in0=u2[:], scalar1=MAGIC, scalar2=MAGIC,
                op0=mybir.AluOpType.add, op1=mybir.AluOpType.subtract)
            fr = small.tile([P, 2, B], F32)
            nc.vector.tensor_sub(fr[:], u2[:], kf[:])
            # sincos[:, 0:32] = sin(yawg), [:, 32:64] = cos(yawg)
            sincos = small.tile([P, 2 * B], F32)
            nc.scalar.activation(out=sincos[:].rearrange("p (c i) -> p c i", c=2),
                                 in_=fr[:],
                                 func=mybir.ActivationFunctionType.Sin,
                                 bias=0.0, scale=2.0 * PI)
            # dummy sqrt so the Sqrt table load happens right after sin in
            # the idle-ACT window instead of on the critical tail. Reading
            # sincos (not ones) keeps the scheduler from hoisting it (and
            # its table load) in front of sin.
            dum = small.tile([P, 1], F32, tag="dum")
            nc.scalar.activation(out=dum[:], in_=sincos[:, 0:1],
                                 func=mybir.ActivationFunctionType.Sqrt)

            # pos_g for both coords: pg[p, c, i], c=0 -> x, 1 -> y
            # m12[p, xsel, c, i] = x_xsel * g_{xsel,c}  in one multiply
            m12 = small.tile([P, 2, 2, B], F32)
            xx = bass.AP(tensor=xta.tensor, offset=x0.offset,
                         ap=[x0.ap[0], [B, 2], [0, 2], [1, B]])
            gAB = bass.AP(tensor=xta.tensor, offset=gA.offset,
                          ap=[gA.ap[0], [2 * B, 2], [B, 2], [1, B]])
            nc.vector.tensor_mul(m12[:], xx, gAB)
            pg = small.tile([P, 2, B], F32)
            nc.vector.tensor_add(pg[:], m12[:, 0], m12[:, 1])
            nc.vector.tensor_add(pg[:], pg[:],
                                 gT.rearrange("p (c i) -> p c i", c=2))

            # CXY[p, c, i, di] = cent_x(i,di) * cs(c,i) + pg(c,i), emitted
            # per scene (inside the sub loop) so the first rect's subs
            # start before the other scenes' cxy is computed
            cxy = singles.tile([P, 2, B, D], F32)

            def emit_cxy(o, s):
                cs_sel = bass.AP(tensor=sincos.tensor,
                                 offset=sincos[:].offset + B + o,
                                 ap=[sincos[:].ap[0], [-B, 2], [1, s], [0, D]])
                cx_rep = bass.AP(tensor=xta.tensor, offset=cxc.offset + o * D,
                                 ap=[cxc.ap[0], [0, 2], [D, s], [1, D]])
                pg_bc = bass.AP(tensor=pg.tensor, offset=pg[:].offset + o,
                                ap=[pg[:].ap[0], [B, 2], [1, s], [0, D]])
                cxy_sl = bass.AP(tensor=cxy.tensor,
                                 offset=cxy[:].offset + o * D,
                                 ap=[cxy[:].ap[0], [B * D, 2], [D, s], [1, D]])
                nc.vector.tensor_mul(cxy_sl, cx_rep, cs_sel)
                nc.vector.tensor_add(cxy_sl, cxy_sl, pg_bc)

            cxyf = cxy[:].rearrange("p c i d -> p (c i d)")
            pap = cxyf.ap[0]
            e = cxyf.ap[-1][0]
            NPTS = B * D

            # doubled per-scene point lists (wrap j = (i+k) mod s becomes a
            # linear read). Kept on DVE: subs read cxy AND cxy2 and may
            # carry only one sync wait, so both must be DVE-written.
            DBL = 2 * NPTS
            cxy2 = singles.tile([P, 2, DBL], F32)
            c2f = cxy2[:].rearrange("p c d -> p (c d)")
            pap2 = c2f.ap[0]
            e2 = c2f.ap[-1][0]
            dbl_off = {}
            do_ = 0
            for (o, s) in scenes:
                dbl_off[o] = do_
                do_ += 2 * D * s

            subx = big.tile([P, NSEG * D], DT_BULK, tag="subx")
            suby = big.tile([P, NSEG * D], DT_BULK, tag="suby")
            sqx = big.tile([P, NSEG * D], DT_BULK, tag="sqx")
            sqy = big.tile([P, NSEG * D], DT_BULK, tag="sqy")
            d2 = big.tile([P, NSEG * D], DT_BULK, tag="d2")
            subt = {0: subx, 1: suby}
            sqt = {0: sqx, 1: sqy}

            def seg_ap(t, off_e, F):
                tf = t[:]
                es = tf.ap[-1][0]
                return bass.AP(tensor=tf.tensor, offset=tf.offset + off_e * es,
                               ap=[tf.ap[0], [es, F]])

            def emit_dbl_copy(o, s, eng):
                # each scene's doubled block is built by the engine that
                # subs it, so the subs see a single-engine cxy/cxy2 pair
                # (DVE) or carry one DVE wait (gpsimd)
                in_ap = bass.AP(tensor=cxyf.tensor,
                                offset=cxyf.offset + o * D * e,
                                ap=[pap, [NPTS * e, 2], [0, 2], [e, D * s]])
                out_ap = bass.AP(tensor=c2f.tensor,
                                 offset=c2f.offset + dbl_off[o] * e2,
                                 ap=[pap2, [DBL * e2, 2], [D * s * e2, 2],
                                     [e2, D * s]])
                eng.tensor_copy(out=out_ap, in_=in_ap)

            def emit_subs(c):
                kind, idx, o, s, K, half, qoff, qn, F = c
                off_e = seg_off[id(c)] * D
                eng = nc.gpsimd if id(c) in gp_subs else nc.vector
                if kind == "A":
                    w = D * K
                    for cc in range(2):
                        a_ap = bass.AP(
                            tensor=cxyf.tensor,
                            offset=cxyf.offset + (cc * NPTS + o * D) * e,
                            ap=[pap, [D * e, s], [e, D], [0, w]])
                        b_ap = bass.AP(
                            tensor=c2f.tensor,
                            offset=c2f.offset + (cc * DBL + dbl_off[o] + D) * e2,
                            ap=[pap2, [D * e2, s], [0, D], [e2, w]])
                        eng.tensor_tensor(
                            out=seg_ap(subt[cc], off_e, F), in0=a_ap,
                            in1=b_ap, op=mybir.AluOpType.subtract)
                else:
                    for cc in range(2):
                        a_ap = bass.AP(
                            tensor=cxyf.tensor,
                            offset=cxyf.offset + (cc * NPTS + o * D) * e,
                            ap=[pap, [D * e, half], [e, D], [0, D]])
                        b_ap = bass.AP(
                            tensor=cxyf.tensor,
                            offset=cxyf.offset + (cc * NPTS + (o + half) * D) * e,
                            ap=[pap, [D * e, half], [0, D], [e, D]])
                        eng.tensor_tensor(
                            out=seg_ap(subt[cc], off_e, F), in0=a_ap,
                            in1=b_ap, op=mybir.AluOpType.subtract)

            # interleave the per-scene cxy, the cxy2 doubling copies, and
            # the subs, in rect (largest-first) order
            done_cxy = set()
            for c in chunks:
                kind, idx, o, s, K, half = c[0], c[1], c[2], c[3], c[4], c[5]
                if o not in done_cxy:
                    emit_cxy(o, s)
                    if kind == "A":
                        emit_dbl_copy(o, s, nc.vector)
                    done_cxy.add(o)
                emit_subs(c)

            # squares per chunk per coord (ACT)
            for c in chunks:
                F = c[8]
                off_e = seg_off[id(c)] * D
                for cc in range(2):
                    nc.scalar.activation(
                        out=seg_ap(sqt[cc], off_e, F),
                        in_=seg_ap(subt[cc], off_e, F),
                        func=mybir.ActivationFunctionType.Square)

            # d2 adds: all-f16 contiguous tensor_tensor hits the HW 2x path
            for c in chunks:
                F = c[8]
                off_e = seg_off[id(c)] * D
                nc.vector.tensor_tensor(out=seg_ap(d2, off_e, F),
                                        in0=seg_ap(sqx, off_e, F),
                                        in1=seg_ap(sqy, off_e, F),
                                        op=mybir.AluOpType.add)

            # ---- min over dj (3-op tt-min tree per group: tensor_reduce
            # never hits the HW fast path, the tree reads fewer elems),
            # then min over di ----
            pdist = singles.tile([P, PP], DT_BULK)
            m2t = big.tile([P, NSEG, 2], DT_BULK, tag="m2")
            m1t = big.tile([P, NSEG], DT_BULK, tag="m1")

            d2f = d2[:]
            ed = d2f.ap[-1][0]
            m2f = m2t[:].rearrange("p a b -> p (a b)")
            em2 = m2f.ap[-1][0]
            m1f = m1t[:]
            em1 = m1f.ap[-1][0]

            def tree_min(u0, un):
                n = un
                in0 = bass.AP(tensor=d2f.tensor,
                              offset=d2f.offset + u0 * D * ed,
                              ap=[d2f.ap[0], [D * ed, n], [ed, 2]])
                in1 = bass.AP(tensor=d2f.tensor,
                              offset=d2f.offset + (u0 * D + 2) * ed,
                              ap=[d2f.ap[0], [D * ed, n], [ed, 2]])
                o2 = bass.AP(tensor=m2f.tensor,
                             offset=m2f.offset + u0 * 2 * em2,
                             ap=[m2f.ap[0], [2 * em2, n], [em2, 2]])
                nc.vector.tensor_tensor(out=o2, in0=in0, in1=in1,
                                        op=mybir.AluOpType.min)
                ia = bass.AP(tensor=m2f.tensor,
                             offset=m2f.offset + u0 * 2 * em2,
                             ap=[m2f.ap[0], [2 * em2, n]])
                ib = bass.AP(tensor=m2f.tensor,
                             offset=m2f.offset + (u0 * 2 + 1) * em2,
                             ap=[m2f.ap[0], [2 * em2, n]])
                o1 = bass.AP(tensor=m1f.tensor, offset=m1f.offset + u0 * em1,
                             ap=[m1f.ap[0], [em1, n]])
                nc.vector.tensor_tensor(out=o1, in0=ia, in1=ib,
                                        op=mybir.AluOpType.min)
                ic = bass.AP(tensor=d2f.tensor,
                             offset=d2f.offset + (u0 * D + 4) * ed,
                             ap=[d2f.ap[0], [D * ed, n]])
                nc.vector.tensor_tensor(out=o1, in0=o1, in1=ic,
                                        op=mybir.AluOpType.min)

            def min2_chunk(c):
                kind, idx, o, s, K, half, qoff, qn, F = c
                u0 = seg_off[id(c)]
                if kind == "A":
                    iv = bass.AP(tensor=m1f.tensor,
                                 offset=m1f.offset + u0 * em1,
                                 ap=[m1f.ap[0], [D * K * em1, s], [em1, K],
                                     [K * em1, D]])
                else:
                    iv = bass.AP(tensor=m1f.tensor,
                                 offset=m1f.offset + u0 * em1,
                                 ap=[m1f.ap[0], [D * em1, half], [em1, D]])
                ov = pdist[:, qoff : qoff + qn]
                nc.vector.tensor_reduce(out=ov, in_=iv,
                                        axis=mybir.AxisListType.X,
                                        op=mybir.AluOpType.min)

            rqt = small.tile([P, PP], F16, tag="rqt")
            rrt = small.tile([P, PP], F16, tag="rrt")
            penn = small.tile([P, PP], F16, tag="penn")
            st_sb = small.tile([PP, NL], F16, tag="st_sb")

            wpos = singles.tile([P, NL], F16)
            mc2 = singles.tile([PP, B], F16)
            tchb = singles.tile([P, 1], F16, tag="tchb")

            staged = []

            def stage_pe_inputs():
                # ACT copies staging every PE operand so each PE op has
                # ACT-only deps (one sync wait); the gpsimd pre-touch of
                # xtb lets the rq multiply (gpsimd) read inv_pd^2 without
                # a DMA wait. Emitted late so the scheduler keeps them off
                # the critical stage-A window.
                nc.scalar.copy(out=wpos[:], in_=xtb[:, XO_W : XO_W + NL])
                nc.scalar.copy(out=mc2[:], in_=mct[:])
                nc.vector.tensor_copy(out=tchb[:], in_=xtb[:, 0:1])
                staged.append(True)

            for gi, g in enumerate(g_chunks):
                u0 = min(seg_off[id(c)] for c in g)
                u1 = max(seg_off[id(c)] + c[8] // D for c in g)
                tree_min(u0, u1 - u0)
                for c in g:
                    min2_chunk(c)
                if not staged:
                    stage_pe_inputs()
                q0, q1 = g_qr[gi]
                # rq = d2min * inv_pd^2; rr = sqrt(rq) = dist/pd;
                # pen = relu(1 - rr)  (all per-pair, f16). rq stays on
                nc.vector.tensor_mul(rqt[:, q0:q1], pdist[:, q0:q1],
                                     prc2[:, q0:q1])
                nc.scalar.activation(out=rrt[:, q0:q1], in_=rqt[:, q0:q1],
                                     func=mybir.ActivationFunctionType.Sqrt)
                nc.scalar.activation(out=penn[:, q0:q1], in_=rrt[:, q0:q1],
                                     func=mybir.ActivationFunctionType.Relu,
                                     bias=1.0, scale=-1.0)
            # ---- tail: S^T = pen^T @ w (pen stationary -> no transpose
            # needed), stage to SBUF, @M (mask+scale folded into M), out.
            # The out DMA issues from the DVE queue right after lout so no
            # cross-engine sem hop sits before the descriptor generation.
            ps2 = psum.tile([PP, NL], F32, tag="ps2")
            nc.tensor.matmul(ps2[:], penn[:], wpos[:], start=True, stop=True)
            nc.vector.tensor_copy(out=st_sb[:], in_=ps2[:])
            ps3 = psum.tile([NL, B], F32, tag="ps3")
            nc.tensor.matmul(ps3[:], st_sb[:], mc2[:], start=True, stop=True)
            lout = small.tile([NL, B], F32, tag="lout")
            nc.vector.tensor_copy(out=lout[:], in_=ps3[:])
            nc.sync.dma_start(out=out[:], in_=lout[:])

    return nc


def _prepare(inputs):
    x = np.ascontiguousarray(inputs["x"], dtype=np.float32)
    extent = np.asarray(inputs["extent"], dtype=np.float32)
    wfa = np.asarray(inputs["world_from_agent"], dtype=np.float32)
    speed = np.asarray(inputs["curr_speed"], dtype=np.float32)
    scene = np.asarray(inputs["scene_index"])

    R = wfa[:, :2, :2]
    tr = wfa[:, :2, 2]
    yaw_off = np.arctan2(R[:, 1, 0], R[:, 0, 0]).astype(np.float32)
    agt_rad = extent[:, 1] / 2.0
    cent_min = -(extent[:, 0] / 2.0) + agt_rad
    cent_max = (extent[:, 0] / 2.0) - agt_rad
    lin = np.linspace(0.0, 1.0, D, dtype=np.float32)
    cent_x = (cent_min[:, None] + (cent_max - cent_min)[:, None] * lin).astype(
        np.float32)
    pd = (agt_rad[:, None] + agt_rad[None, :] + BUFFER_DIST).astype(np.float32)
    moving = (np.abs(speed) > SPEED_TH)

    # contiguous scene blocks (scene_index is sorted)
    _, starts, counts = np.unique(scene, return_index=True, return_counts=True)
    scenes = [(int(o), int(s)) for o, s in zip(starts, counts)]
    assert sum(s for _, s in scenes) == B
    for o, s in scenes:
        assert (scene[o : o + s] == scene[o]).all()

    pairs_i = []
    pairs_j = []
    for (o, s, K, half) in _rects(scenes):
        for i in range(s):
            for k in range(1, K + 1):
                pairs_i.append(o + i)
                pairs_j.append(o + (i + k) % s)
        for i in range(half):
            pairs_i.append(o + i)
            pairs_j.append(o + i + half)
    pairs_i = np.array(pairs_i, dtype=np.int64)
    pairs_j = np.array(pairs_j, dtype=np.int64)
    PP = len(pairs_i)
    inv_pd2 = (1.0 / pd[pairs_i, pairs_j] ** 2).astype(np.float32)

    # pair -> agent incidence matrix for the tail matmul, with the moving
    # mask and the 1/WSCALE de-scaling folded into its columns
    mcol = (moving.astype(np.float32) / WSCALE)
    mcon = np.zeros((PP, B), dtype=np.float32)
    mcon[np.arange(PP), pairs_i] = mcol[pairs_i]
    mcon[np.arange(PP), pairs_j] = mcol[pairs_j]
    mcon = mcon.astype(np.float16)

    twopi = 2.0 * np.pi
    consts = np.concatenate([
        np.concatenate([R[:, 0, 0], R[:, 1, 0],          # gA
                        R[:, 0, 1], R[:, 1, 1]]).astype(np.float16),
        np.concatenate([tr[:, 0], tr[:, 1]]).astype(np.float32)
          .view(np.float16),                             # gT (f32 island)
        np.concatenate([2.0 + yaw_off / twopi, 2.25 + yaw_off / twopi])
          .astype(np.float32).view(np.float16),          # shifts2 (f32)
        cent_x.reshape(-1).astype(np.float16),
    ])
    assert consts.shape[0] == XWA - 3 * B

    w = DECAY_RATE ** np.arange(T, dtype=np.float32)
    w = w / w.sum()
    wz = np.zeros((P, NL), dtype=np.float32)
    for nl in range(NL):
        wz[nl * T : (nl + 1) * T, nl] = w / B * WSCALE

    ident = np.zeros((P, NL), dtype=np.float32)
    for nl in range(NL):
        ident[nl, nl] = 1.0

    XWB = _xin_width_b(PP)
    xinb_row = np.empty((P, XWB), dtype=np.float32)
    xinb_row[:, XO_MVR : XO_MVR + B] = (moving.astype(np.float32)
                                        / WSCALE)[None, :]
    xinb_row[:, XO_W : XO_W + NL] = wz
    xinb_row[:, XO_ID : XO_ID + NL] = ident
    xinb_row[:, XO_PRC : XO_PRC + PP] = inv_pd2[None, :]
    xinb_row = xinb_row.astype(np.float16)

    in_maps = []
    for c in range(NCORES):
        xs = x[:, c * NL : (c + 1) * NL, :, :]          # (B, NL, T, 6)
        xs = xs[..., [0, 1, 3]]                          # (B, NL, T, 3)
        xdat = xs.transpose(1, 2, 3, 0).reshape(P, 3 * B).astype(np.float16)
        xina = np.empty((P, XWA), dtype=np.float16)
        xina[:, 0 : 3 * B] = xdat
        xina[:, 3 * B :] = consts[None, :]
        in_maps.append({"xina": xina, "xinb": xinb_row, "mcon": mcon})

    return scenes, PP, in_maps, moving


_CACHE = {}


def _get_nc(scenes, PP):
    key = (tuple(scenes), PP)
    if key not in _CACHE:
        _CACHE[key] = _build_nc(scenes, PP)
    return _CACHE[key]


def _run(inputs, trace=False):
    scenes, PP, in_maps, moving = _prepare(inputs)
    nc = _get_nc(scenes, PP)
    res = run_bass_kernel_spmd(nc, in_maps, core_ids=list(range(NCORES)),
                               trace=trace)
    outf = np.zeros((B, N), dtype=np.float32)
    for c in range(NCORES):
        lc = res.results[c]["loss"]                      # (NL, B)
        for nl in range(NL):
            outf[:, c * NL + nl] = lc[nl]
    return outf, res


def kernel(**inputs):
    outf, _ = _run(inputs, trace=False)
    return outf


def _ensure_ntff_hook():
    """Register the axon NTFF profile hook if the container's antenv lacks it."""
    try:
        from antenv.axon_hooks import get_axon_ntff_profile_hook  # noqa: F401
        return
    except ImportError:
        pass
    import types

    if "/root/.axon_site" not in sys.path:
        sys.path.insert(0, "/root/.axon_site")
    from trn_agent_boot.trn_boot import _ntff_profile_via_ctypes

    hook = _ntff_profile_via_ctypes("/opt/axon/libaxon_pjrt.so")
    mod = types.ModuleType("antenv.axon_hooks")
    mod.get_axon_ntff_profile_hook = lambda: hook
    mod.set_axon_ntff_profile_hook = lambda h: None
    sys.modules["antenv.axon_hooks"] = mod


def run_traced(inputs):
    """Correctness output + profiled exec time (ns) via NTFF trace."""
    _ensure_ntff_hook()
    outf, res = _run(inputs, trace=True)
    return outf, res.exec_time_ns
